# revision 1
# baseline (speedup 1.0000x reference)
"""GCN forward on 8 TRN2 NeuronCores — host prep + Bass/Tile builder + runner.

Model (reference.py): 2-layer GCN, N=100000 nodes, E=1600000 edges,
IN=HID=128, OUT=64, symmetric normalization with self-loops.

Math:
  deg[t] = in_degree(t)+1 ; dinv = deg^-1/2
  table1[s] = dinv[s]*(x@W1)[s] ; y1[t] = relu(dinv[t]*sum_e table1[row_e] + b1)
  table2[s] = dinv[s]*y1[s]     ; out[t] = (dinv[t]*sum_e table2[row_e])@W2 + b2
"""

import sys

sys.path.insert(0, "/opt/trn_rl_repo")
import numpy as np
import ml_dtypes

import concourse.bass as bass
import concourse.mybir as mybir
import concourse.tile as tile
from concourse import bacc
from concourse.bass_utils import run_bass_kernel_spmd

F32 = mybir.dt.float32
BF16 = mybir.dt.bfloat16
I16 = mybir.dt.int16
I32 = mybir.dt.int32
AF = mybir.ActivationFunctionType
ALU = mybir.AluOpType

P = 128
N, E = 100000, 1600000
IN, HID, OUT = 128, 128, 64
NCORES = 8
TPC = 12500
PPC = 12544
NPAD = PPC * NCORES
NCHUNK = 4
CHUNK = NPAD // NCHUNK
NG = PPC // P
SG = 12
MAX_CALL_BLOCKS = 48


def host_prep(edge_index):
    row = np.asarray(edge_index[0], dtype=np.int64)
    col = np.asarray(edge_index[1], dtype=np.int64)
    deg = np.bincount(col, minlength=N).astype(np.int64) + 1

    n_all = np.arange(N, dtype=np.int64)
    gids = (n_all // TPC) * PPC + (n_all % TPC)
    grow = np.concatenate([gids[row], gids])
    gcol = np.concatenate([gids[col], gids])

    owner = gcol // PPC
    g_all = (gcol - owner * PPC) >> 7
    loc_all = (gcol - owner * PPC) & 127
    ch_all = grow // CHUNK
    lidx_all = (grow % CHUNK).astype(np.int16)

    NSEG = NG * NCHUNK
    counts = np.zeros((NCORES, NSEG), np.int64)
    per_core = []
    for c in range(NCORES):
        sel = owner == c
        g = g_all[sel]
        ch = ch_all[sel]
        order = np.lexsort((ch, g))
        seg = (g * NCHUNK + ch)[order]
        counts[c] = np.bincount(seg, minlength=NSEG)
        per_core.append((seg, lidx_all[sel][order], loc_all[sel][order]))

    nb_seg = -(-counts.max(axis=0) // P)
    nb_seg = np.maximum(nb_seg, (np.arange(NSEG) % NCHUNK == 0).astype(np.int64))
    NB = nb_seg.reshape(NG, NCHUNK)

    sgs = []
    g0 = 0
    while g0 < NG:
        sgs.append((g0, min(SG, NG - g0)))
        g0 += SG
    seg_base = np.zeros(NSEG, np.int64)
    calls = []
    tot = 0
    for sgi, (gs, ng) in enumerate(sgs):
        for ch in range(NCHUNK):
            call_start = tot
            for gl in range(ng):
                g = gs + gl
                seg_base[g * NCHUNK + ch] = tot
                tot += NB[g, ch]
            nb_call = tot - call_start
            off = call_start
            while nb_call > 0:
                take = min(nb_call, MAX_CALL_BLOCKS)
                calls.append((sgi, ch, off, take))
                off += take
                nb_call -= take
    TOTB = tot

    idx_list, colv_list = [], []
    for c in range(NCORES):
        seg, lidx, loc = per_core[c]
        seg_start = np.zeros(NSEG, np.int64)
        seg_start[1:] = np.cumsum(counts[c])[:-1]
        rank = np.arange(seg.shape[0]) - seg_start[seg]
        dest = seg_base[seg] * P + rank
        idx_arr = np.zeros(TOTB * P, np.int16)
        colv_arr = np.full(TOTB * P, -1.0, np.float32)
        idx_arr[dest] = lidx
        colv_arr[dest] = loc.astype(np.float32)
        idx_list.append(np.tile(idx_arr.reshape(-1, 16).T, (8, 1)).copy())
        colv_list.append(
            np.ascontiguousarray(colv_arr.reshape(TOTB, P).T.astype(ml_dtypes.bfloat16))
        )

    degp = np.ones(NPAD, np.int32)
    degp[gids] = deg.astype(np.int32)
    dego_list = [
        np.ascontiguousarray(degp[c * PPC : (c + 1) * PPC].reshape(NG, P).T)
        for c in range(NCORES)
    ]
    # blockmap: blk -> (gl_in_sg, is_first, is_last)
    last_ch = [int(np.nonzero(NB[g])[0][-1]) for g in range(NG)]
    blockmap = {}
    for sgi, (gs, ng) in enumerate(sgs):
        for ch in range(NCHUNK):
            for gl in range(ng):
                g = gs + gl
                base = int(seg_base[g * NCHUNK + ch])
                nbg = int(NB[g, ch])
                for k in range(nbg):
                    blockmap[base + k] = (
                        gl,
                        ch == 0 and k == 0,
                        ch == last_ch[g] and k == nbg - 1,
                    )
    sched = {
        "NB": NB,
        "sgs": sgs,
        "calls": calls,
        "TOTB": TOTB,
        "seg_base": seg_base,
        "blockmap": blockmap,
    }
    return sched, idx_list, colv_list, dego_list, gids


def build_kernel(sched, stage=9):
    NB, sgs, calls, TOTB = sched["NB"], sched["sgs"], sched["calls"], sched["TOTB"]
    blockmap = sched["blockmap"]

    nc = bacc.Bacc("TRN2", target_bir_lowering=False, num_devices=NCORES)
    xT = nc.dram_tensor("xT", [P, PPC], F32, kind="ExternalInput")
    dego = nc.dram_tensor("dego", [P, NG], I32, kind="ExternalInput")
    W1 = nc.dram_tensor("W1", [IN, HID], F32, kind="ExternalInput")
    W2 = nc.dram_tensor("W2", [HID, OUT], F32, kind="ExternalInput")
    b1r = nc.dram_tensor("b1r", [P, HID], F32, kind="ExternalInput")
    b2r = nc.dram_tensor("b2r", [P, OUT], F32, kind="ExternalInput")
    iota = nc.dram_tensor("iota", [P, P], BF16, kind="ExternalInput")
    idx = nc.dram_tensor("idx", [P, TOTB * 8], I16, kind="ExternalInput")
    colv = nc.dram_tensor("colv", [P, TOTB], BF16, kind="ExternalInput")
    y = nc.dram_tensor("y", [P, NG, OUT], F32, kind="ExternalOutput")

    with tile.TileContext(nc) as tc:
        with (
            tc.tile_pool(name="const", bufs=1) as cpool,
            tc.tile_pool(name="sb", bufs=2) as sb,
            tc.tile_pool(name="dram", bufs=1, space="DRAM") as dpool,
            tc.tile_pool(name="psX", bufs=2, space="PSUM") as psX,
            tc.tile_pool(name="psAgg", bufs=2, space="PSUM") as psAgg,
        ):
            # ---- constants ----
            W1b = cpool.tile([IN, HID], BF16)
            nc.gpsimd.dma_start(out=W1b[:], in_=W1[:])
            W2b = cpool.tile([HID, OUT], BF16)
            nc.gpsimd.dma_start(out=W2b[:], in_=W2[:])
            b1t = cpool.tile([P, HID], F32)
            nc.sync.dma_start(out=b1t[:], in_=b1r[:])
            b2t = cpool.tile([P, OUT], F32)
            nc.sync.dma_start(out=b2t[:], in_=b2r[:])
            iota_t = cpool.tile([P, P], BF16)
            nc.sync.dma_start(out=iota_t[:], in_=iota[:])
            idx_t = cpool.tile([P, TOTB * 8], I16)
            nc.sync.dma_start(out=idx_t[:], in_=idx[:])
            colv_t = cpool.tile([P, TOTB], BF16)
            nc.sync.dma_start(out=colv_t[:], in_=colv[:])

            dego_i = cpool.tile([P, NG], I32)
            nc.sync.dma_start(out=dego_i[:], in_=dego[:])
            dego_f = cpool.tile([P, NG], F32)
            nc.vector.tensor_copy(out=dego_f[:], in_=dego_i[:])
            dsq = cpool.tile([P, NG], F32)
            nc.scalar.activation(dsq[:], dego_f[:], AF.Sqrt)
            dinv = cpool.tile([P, NG], F32)
            nc.vector.reciprocal(out=dinv[:], in_=dsq[:])

            # ---- phase A ----
            t1in = dpool.tile([PPC, HID], BF16)
            table1 = dpool.tile([NPAD, HID], BF16)
            if stage >= 1:
                XC = 14
                for c0 in range(0, NG, XC):
                    xbf = sb.tile([P, XC * P], BF16, tag="xbf", name="xbf")
                    nc.gpsimd.dma_start(out=xbf[:], in_=xT[:, c0 * P : (c0 + XC) * P])
                    t1s = sb.tile([P, XC, HID], BF16, tag="t1s", name="t1s")
                    for b in range(XC):
                        ps = psX.tile([P, HID], F32, tag="px", name="ps")
                        nc.tensor.matmul(
                            out=ps[:],
                            lhsT=xbf[:, b * P : (b + 1) * P],
                            rhs=W1b[:],
                            start=True,
                            stop=True,
                        )
                        nc.scalar.activation(
                            t1s[:, b, :], ps[:], AF.Copy,
                            scale=dinv[:, c0 + b : c0 + b + 1],
                        )
                    nc.sync.dma_start(
                        out=t1in.rearrange("(n p) f -> p n f", p=P)[:, c0 : c0 + XC, :],
                        in_=t1s[:],
                    )
            if stage >= 2:
                nc.gpsimd.collective_compute(
                    "AllGather", ALU.bypass, ins=[t1in[:]], outs=[table1[:]],
                    replica_groups=[list(range(NCORES))],
                )

            agin = dpool.tile([PPC, HID], BF16)
            table2 = dpool.tile([NPAD, HID], BF16)

            def agg_layer(table, orientation, epilogue, nsg):
                for sgi, (gs, ng) in enumerate(sgs[:nsg]):
                    nbank = -(-ng // 4)
                    banks = [
                        psAgg.tile([P, 512], F32, name=f"bank{i}", tag=f"aggbank{i}", bufs=2)
                        for i in range(nbank)
                    ]
                    for bk in banks:
                        nc.vector.memset(bk[:], 0.0)
                    for ch in range(NCHUNK):
                        for (csgi, cch, boff, nb) in calls:
                            if csgi != sgi or cch != ch:
                                continue
                            msgs = sb.tile([P, nb, HID], BF16, tag="msgs", bufs=3, name="msgs")
                            nc.gpsimd.dma_gather(
                                msgs[:],
                                table[ch * CHUNK : (ch + 1) * CHUNK, :],
                                idx_t[:, boff * 8 : (boff + nb) * 8],
                                nb * P,
                                nb * P,
                                HID,
                                single_packet=False,
                            )
                            S = sb.tile([P, nb, P], BF16, tag="S", bufs=3, name="S")
                            nc.vector.tensor_tensor(
                                out=S[:],
                                in0=colv_t[:, boff : boff + nb, None].to_broadcast([P, nb, P]),
                                in1=iota_t[:, None, :].to_broadcast([P, nb, P]),
                                op=ALU.is_equal,
                            )
                            for k in range(nb):
                                blk = boff + k
                                gl, is_first, is_last = blockmap[blk]
                                region = banks[gl // 4][:, (gl % 4) * P : (gl % 4 + 1) * P]
                                if orientation == 1:
                                    nc.tensor.matmul(
                                        out=region, lhsT=S[:, k, :], rhs=msgs[:, k, :],
                                        start=False, stop=is_last, skip_group_check=True,
                                    )
                                else:
                                    nc.tensor.matmul(
                                        out=region, lhsT=msgs[:, k, :], rhs=S[:, k, :],
                                        start=False, stop=is_last, skip_group_check=True,
                                    )
                    for gl in range(ng):
                        epilogue(
                            sgi, gs + gl, gl, ng,
                            banks[gl // 4][:, (gl % 4) * P : (gl % 4 + 1) * P],
                        )

            # ---- L1 ----
            y1sg = {}

            def epi1(sgi, g, gl, ng, region):
                if gl == 0:
                    y1sg[sgi] = sb.tile([P, ng, HID], BF16, name="y1s", tag="y1s", bufs=2)
                tmp = sb.tile([P, HID], F32, tag="epi1a", bufs=2, name="tmp")
                nc.scalar.activation(tmp[:], region, AF.Copy, scale=dinv[:, g : g + 1])
                tmp2 = sb.tile([P, HID], F32, tag="epi1b", bufs=2, name="tmp2")
                nc.vector.tensor_tensor(out=tmp2[:], in0=tmp[:], in1=b1t[:], op=ALU.add)
                nc.vector.tensor_scalar(
                    out=y1sg[sgi][:, gl, :], in0=tmp2[:],
                    scalar1=0.0, scalar2=dinv[:, g : g + 1],
                    op0=ALU.max, op1=ALU.mult,
                )
                if gl == ng - 1:
                    gs = g - gl
                    nc.sync.dma_start(
                        out=agin.rearrange("(n p) f -> p n f", p=P)[:, gs : gs + ng, :],
                        in_=y1sg[sgi][:],
                    )

            if stage >= 3:
                agg_layer(table1, 1, epi1, nsg=1 if stage == 3 else len(sgs))
            if stage >= 5:
                nc.gpsimd.collective_compute(
                    "AllGather", ALU.bypass, ins=[agin[:]], outs=[table2[:]],
                    replica_groups=[list(range(NCORES))],
                )

            # ---- L2 ----
            outsg = {}

            def epi2(sgi, g, gl, ng, region):
                if gl == 0:
                    outsg[sgi] = sb.tile([P, ng, OUT], F32, name="outs", tag="outs", bufs=2)
                a2 = sb.tile([HID, P], BF16, tag="a2", bufs=2, name="a2")
                nc.vector.tensor_copy(out=a2[:], in_=region)
                psf = psX.tile([P, OUT], F32, tag="px", name="psf", bufs=2)
                nc.tensor.matmul(out=psf[:], lhsT=a2[:], rhs=W2b[:], start=True, stop=True)
                tmp = sb.tile([P, OUT], F32, tag="epi2a", bufs=2, name="tmp3")
                nc.scalar.activation(tmp[:], psf[:], AF.Copy, scale=dinv[:, g : g + 1])
                nc.vector.tensor_tensor(
                    out=outsg[sgi][:, gl, :], in0=tmp[:], in1=b2t[:], op=ALU.add
                )
                if gl == ng - 1:
                    gs = g - gl
                    nc.sync.dma_start(out=y[:, gs : gs + ng, :], in_=outsg[sgi][:])

            if stage >= 6:
                agg_layer(table2, 2, epi2, nsg=1 if stage == 6 else len(sgs))

    nc.finalize()
    return nc


def make_in_maps(inputs, sched, idx_list, colv_list, dego_list):
    x = np.asarray(inputs["x"], np.float32)
    W1 = np.asarray(inputs["W1"], np.float32)
    W2 = np.asarray(inputs["W2"], np.float32)
    b1 = np.asarray(inputs["b1"], np.float32)
    b2 = np.asarray(inputs["b2"], np.float32)
    iota_np = np.tile(np.arange(P, dtype=ml_dtypes.bfloat16)[None, :], (P, 1))
    b1r = np.tile(b1[None, :], (P, 1)).astype(np.float32)
    b2r = np.tile(b2[None, :], (P, 1)).astype(np.float32)
    in_maps = []
    for c in range(NCORES):
        xs = np.zeros((P, PPC), np.float32)
        xs[:, :TPC] = x[c * TPC : (c + 1) * TPC].T
        in_maps.append(
            {
                "xT": xs,
                "dego": dego_list[c],
                "W1": W1,
                "W2": W2,
                "b1r": b1r,
                "b2r": b2r,
                "iota": iota_np,
                "idx": idx_list[c],
                "colv": colv_list[c],
            }
        )
    return in_maps


def assemble_output(results):
    outs = []
    for c in range(NCORES):
        yc = results[c]["y"]
        yc = np.transpose(yc, (1, 0, 2)).reshape(PPC, OUT)[:TPC]
        outs.append(yc)
    return np.concatenate(outs, axis=0)


def kernel(**inputs):
    sched, idx_list, colv_list, dego_list, _ = host_prep(inputs["edge_index"])
    nc = build_kernel(sched)
    in_maps = make_in_maps(inputs, sched, idx_list, colv_list, dego_list)
    res = run_bass_kernel_spmd(nc, in_maps, core_ids=list(range(NCORES)))
    return assemble_output(res.results)



# revision 13
# speedup vs baseline: 11.1809x; 11.1809x over previous
"""GCN forward on 8 TRN2 NeuronCores — slot-aligned gather + strided-reduce design.

Model (reference.py): 2-layer GCN, N=100000 nodes, E=1600000 edges,
IN=HID=128, OUT=64, symmetric normalization with self-loops.

Math (dinv = (in_deg+1)^-1/2, folded on host where linear):
  table1[s] = (dinv*x)[s] @ W1            (x pre-scaled on host)
  y1[t]     = relu(dinv[t] * sum_e table1[src_e] + b1)
  table2[s] = dinv[s] * y1[s]
  out[t]    = dinv[t] * (sum_e table2[src_e]) @ W2 + b2

Device layout: targets of each core are permuted (degree-balanced) into
(group g in [0,98), slot s in [0,128)) positions. The per-edge gather
stream is slot-aligned — the edge for slot s sits at stream position
≡ s (mod 128) — so aggregation is a single strided tensor_reduce per
slab of groups instead of per-block one-hot matmuls.
"""

import sys

sys.path.insert(0, "/opt/trn_rl_repo")
import numpy as np
import ml_dtypes

import concourse.bass as bass
import concourse.mybir as mybir
import concourse.tile as tile
from concourse import bacc
from concourse.bass_utils import run_bass_kernel_spmd

F32 = mybir.dt.float32
BF16 = mybir.dt.bfloat16
I16 = mybir.dt.int16
AF = mybir.ActivationFunctionType
ALU = mybir.AluOpType

P = 128
N, E = 100000, 1600000
IN, HID, OUT = 128, 128, 64
NCORES = 8
TPC = 12500
PPC = 12544
NPAD = PPC * NCORES
NCHUNK = 4
CHUNK = NPAD // NCHUNK  # 25088 rows per gather window (int16 index range)
NG = PPC // P  # 98 target groups per core
GSLAB = 3  # groups per reduce slab
PADIDX = 12500  # a guaranteed-zero table row inside every chunk


def host_prep(edge_index):
    row = np.asarray(edge_index[0], dtype=np.int64)
    col = np.asarray(edge_index[1], dtype=np.int64)
    loop = np.arange(N, dtype=np.int64)
    src = np.concatenate([row, loop])
    tgt = np.concatenate([col, loop])
    deg = np.bincount(tgt, minlength=N)
    dinv = 1.0 / np.sqrt(deg.astype(np.float64))
    dinv = dinv.astype(np.float32)

    # per-core degree-balanced position assignment: node (core c, local r)
    # -> position posmap[c][r] in [0, TPC); similar-degree targets share a
    # group so per-slot edge counts stay even.
    posmap = np.zeros((NCORES, TPC), np.int64)
    for c in range(NCORES):
        degc = deg[c * TPC : (c + 1) * TPC]
        order = np.argsort(-degc, kind="stable")
        posmap[c][order] = np.arange(TPC)

    c_t = tgt // TPC
    pos_t = posmap[c_t, tgt % TPC]
    g_t = pos_t >> 7
    s_t = pos_t & 127

    c_s = src // TPC
    gsrc = c_s * PPC + posmap[c_s, src % TPC]
    chv = gsrc // CHUNK
    lidx = (gsrc % CHUNK).astype(np.int16)

    key = ((c_t * NG + g_t) * NCHUNK + chv) * P + s_t
    cnt = np.bincount(key, minlength=NCORES * NG * NCHUNK * P).reshape(
        NCORES, NG, NCHUNK, P
    )

    slabs = [(i, min(GSLAB, NG - i)) for i in range(0, NG, GSLAB)]
    nbu = np.zeros(len(slabs), np.int64)
    slaboff = np.zeros(len(slabs), np.int64)
    si_of_g = np.zeros(NG, np.int64)
    gl_of_g = np.zeros(NG, np.int64)
    off = 0
    for si, (gs0, ng) in enumerate(slabs):
        nbu[si] = max(1, int(cnt[:, gs0 : gs0 + ng].max()))
        slaboff[si] = off
        off += NCHUNK * ng * nbu[si] * P
        si_of_g[gs0 : gs0 + ng] = si
        gl_of_g[gs0 : gs0 + ng] = np.arange(ng)
    TOTP = off

    # rank of each edge within its (core, g, ch, s) bucket
    order_e = np.argsort(key, kind="stable")
    ks = key[order_e]
    starts = np.zeros(NCORES * NG * NCHUNK * P, np.int64)
    flat_cnt = cnt.reshape(-1)
    starts[1:] = np.cumsum(flat_cnt)[:-1]
    rank = np.arange(ks.shape[0]) - starts[ks]

    si_e = si_of_g[g_t[order_e]]
    ng_e = np.array([s[1] for s in slabs], np.int64)[si_e]
    pos = (
        slaboff[si_e]
        + ((chv[order_e] * ng_e + gl_of_g[g_t[order_e]]) * nbu[si_e] + rank) * P
        + s_t[order_e]
    )
    core_e = c_t[order_e]
    lidx_e = lidx[order_e]

    idx_list = []
    for c in range(NCORES):
        arr = np.zeros(TOTP, np.int16)
        # default pad: PADIDX (zero row of the chunk)
        arr[:] = PADIDX
        sel = core_e == c
        arr[pos[sel]] = lidx_e[sel]
        idx_list.append(np.tile(arr.reshape(-1, 16).T, (8, 1)).copy())

    sched = {
        "slabs": slabs,
        "nbu": nbu,
        "slaboff": slaboff,
        "TOTP": TOTP,
        "posmap": posmap,
        "dinv": dinv,
    }
    return sched, idx_list, None, None, None


def build_kernel(sched, stage=9, reps=1, shared_ag=True, drop=frozenset()):
    slabs, nbu, slaboff, TOTP = (
        sched["slabs"],
        sched["nbu"],
        sched["slaboff"],
        sched["TOTP"],
    )
    drop = frozenset(drop)

    nc = bacc.Bacc("TRN2", target_bir_lowering=False, num_devices=NCORES)
    xpT = nc.dram_tensor("xpT", [P, PPC], BF16, kind="ExternalInput")
    W1i = nc.dram_tensor("W1i", [IN, HID], BF16, kind="ExternalInput")
    W2i = nc.dram_tensor("W2i", [HID, OUT], BF16, kind="ExternalInput")
    b1i = nc.dram_tensor("b1i", [P, HID], F32, kind="ExternalInput")
    b2i = nc.dram_tensor("b2i", [OUT, 1], F32, kind="ExternalInput")
    dsgi = nc.dram_tensor("dsgi", [P, NG], F32, kind="ExternalInput")
    drwi = nc.dram_tensor("drwi", [P, PPC], BF16, kind="ExternalInput")
    idx = nc.dram_tensor("idx", [P, TOTP // 16], I16, kind="ExternalInput")
    y = nc.dram_tensor("y", [OUT, PPC], F32, kind="ExternalOutput")

    with tile.TileContext(nc) as tc:
        with (
            tc.tile_pool(name="const", bufs=1) as cpool,
            tc.tile_pool(name="sb", bufs=2) as sb,
            tc.tile_pool(name="dram", bufs=1, space="DRAM") as dpool,
            tc.tile_pool(name="psX", bufs=2, space="PSUM") as psX,
        ):
            # ---- constants (loaded once) ----
            W1b = cpool.tile([IN, HID], BF16)
            nc.sync.dma_start(out=W1b[:], in_=W1i[:])
            W2b = cpool.tile([HID, OUT], BF16)
            nc.sync.dma_start(out=W2b[:], in_=W2i[:])
            b1t = cpool.tile([P, HID], F32)
            nc.sync.dma_start(out=b1t[:], in_=b1i[:])
            b2c = cpool.tile([OUT, 1], F32)
            nc.sync.dma_start(out=b2c[:], in_=b2i[:])
            dsg = cpool.tile([P, NG], F32)
            nc.sync.dma_start(out=dsg[:], in_=dsgi[:])
            dsgb = cpool.tile([P, NG], BF16)
            nc.vector.tensor_copy(out=dsgb[:], in_=dsg[:])
            drw = cpool.tile([P, PPC], BF16)
            nc.sync.dma_start(out=drw[:], in_=drwi[:])

            use_shared = shared_ag and "ag" not in drop
            addr_space = "Shared" if use_shared else "Local"
            t1in = dpool.tile([PPC, HID], BF16)
            t2in = dpool.tile([PPC, HID], BF16)
            tables1 = [
                dpool.tile([NPAD, HID], BF16, addr_space=addr_space, name=f"tb1_{r}")
                for r in range(reps if use_shared else 1)
            ]
            tables2 = [
                dpool.tile([NPAD, HID], BF16, addr_space=addr_space, name=f"tb2_{r}")
                for r in range(reps if use_shared else 1)
            ]

            if drop:
                ini = sb.tile([P, NG, P], BF16, tag="ini", bufs=1, name="ini")
                nc.vector.memset(ini[:], 0.0)
                for t in (t1in, t2in):
                    nc.sync.dma_start(
                        out=t.rearrange("(n p) f -> p n f", p=P)[:, :, :], in_=ini[:]
                    )
                if "ag" in drop:
                    for t in (tables1[0], tables2[0]):
                        for j in range(NCORES):
                            nc.sync.dma_start(
                                out=t.rearrange(
                                    "(c n p) f -> c p n f", c=NCORES, p=P
                                )[j],
                                in_=ini[:],
                            )

            def agg(table, acc, transpose):
                for si, (gs0, ng) in enumerate(slabs):
                    nb = int(nbu[si])
                    K = ng * nb * P
                    idxs = sb.tile(
                        [P, NCHUNK * K // 16], I16, tag="idxs", bufs=1, name="idxs"
                    )
                    nc.sync.dma_start(
                        out=idxs[:],
                        in_=idx[:, slaboff[si] // 16 : (slaboff[si] + NCHUNK * K) // 16],
                    )
                    if transpose:
                        msgs = sb.tile(
                            [P, NCHUNK * K], BF16, tag="msgs", bufs=1, name="msgsT"
                        )
                    else:
                        msgs = sb.tile(
                            [P, NCHUNK * ng * nb, HID], BF16, tag="msgs", bufs=1,
                            name="msgs",
                        )
                    if "gather" not in drop:
                        for ch in range(NCHUNK):
                            if transpose:
                                o = msgs[:, None, ch * K : (ch + 1) * K]
                            else:
                                o = msgs[:, ch * ng * nb : (ch + 1) * ng * nb, :]
                            nc.gpsimd.dma_gather(
                                o,
                                table[ch * CHUNK : (ch + 1) * CHUNK, :],
                                idxs[:, ch * K // 16 : (ch + 1) * K // 16],
                                K,
                                K,
                                HID,
                                transpose=transpose,
                                single_packet=False,
                            )
                    elif si == 0:
                        nc.vector.memset(msgs[:], 0.0)
                    if "reduce" in drop:
                        continue
                    if transpose:
                        v = msgs.rearrange(
                            "p (c g b s) -> p g s c b", c=NCHUNK, g=ng, b=nb, s=P
                        )
                    else:
                        v = msgs.rearrange(
                            "p (c g b) f -> p g f c b", c=NCHUNK, g=ng, b=nb
                        )
                    nc.vector.tensor_reduce(
                        out=acc[:, gs0 : gs0 + ng, :],
                        in_=v,
                        axis=mybir.AxisListType.XY,
                        op=ALU.add,
                    )

            def body(r):
                table1 = tables1[r if use_shared else 0]
                table2 = tables2[r if use_shared else 0]

                # ---- phase A: table1 = xpre @ W1, node-major rows ----
                if stage >= 1:
                    XC = 14
                    for c0 in range(0, NG, XC):
                        xbf = sb.tile([P, XC * P], BF16, tag="xbf", name="xbf")
                        nc.sync.dma_start(
                            out=xbf[:], in_=xpT[:, c0 * P : (c0 + XC) * P]
                        )
                        t1s = sb.tile([P, XC, HID], BF16, tag="t1s", name="t1s")
                        for h0 in range(0, XC, 4):
                            hn = min(4, XC - h0)
                            ps = psX.tile([P, hn * P], F32, tag="px", name="ps")
                            for b in range(hn):
                                nc.tensor.matmul(
                                    out=ps[:, b * P : (b + 1) * P],
                                    lhsT=xbf[:, (h0 + b) * P : (h0 + b + 1) * P],
                                    rhs=W1b[:],
                                    start=True,
                                    stop=True,
                                )
                            nc.scalar.activation(
                                t1s[:, h0 : h0 + hn, :], ps[:], AF.Copy
                            )
                        nc.sync.dma_start(
                            out=t1in.rearrange("(n p) f -> p n f", p=P)[
                                :, c0 : c0 + XC, :
                            ],
                            in_=t1s[:],
                        )
                if stage >= 2 and "ag" not in drop:
                    nc.gpsimd.collective_compute(
                        "AllGather", ALU.bypass, ins=[t1in[:]], outs=[table1[:]],
                        replica_groups=[list(range(NCORES))],
                    )

                # ---- L1 aggregation (slot-major acc1) + epilogue ----
                if stage >= 3:
                    acc1 = sb.tile([P, NG, HID], F32, tag="acc", bufs=1, name="acc1")
                    agg(table1, acc1, transpose=False)
                    if "epi" not in drop:
                        nc.vector.tensor_tensor(
                            out=acc1[:], in0=acc1[:],
                            in1=dsg[:, :, None].to_broadcast([P, NG, HID]),
                            op=ALU.mult,
                        )
                        nc.vector.tensor_tensor(
                            out=acc1[:], in0=acc1[:],
                            in1=b1t[:, None, :].to_broadcast([P, NG, HID]),
                            op=ALU.add,
                        )
                        y1t = sb.tile([P, NG, HID], BF16, tag="bfb", bufs=1, name="y1t")
                        nc.scalar.activation(y1t[:], acc1[:], AF.Relu)
                        nc.vector.tensor_tensor(
                            out=y1t[:], in0=y1t[:],
                            in1=dsgb[:, :, None].to_broadcast([P, NG, HID]),
                            op=ALU.mult,
                        )
                        nc.sync.dma_start(
                            out=t2in.rearrange("(n p) f -> p n f", p=P)[:, :, :],
                            in_=y1t[:],
                        )
                if stage >= 5 and "ag" not in drop:
                    nc.gpsimd.collective_compute(
                        "AllGather", ALU.bypass, ins=[t2in[:]], outs=[table2[:]],
                        replica_groups=[list(range(NCORES))],
                    )

                # ---- L2 aggregation (feature-major acc2) + projection ----
                if stage >= 6:
                    acc2 = sb.tile([P, NG, P], F32, tag="acc", bufs=1, name="acc2")
                    agg(table2, acc2, transpose=True)
                    if "epi" in drop:
                        return
                    a2s = sb.tile([P, PPC], BF16, tag="bfb", bufs=1, name="a2s")
                    nc.vector.tensor_tensor(
                        out=a2s[:],
                        in0=acc2.rearrange("p g s -> p (g s)"),
                        in1=drw[:],
                        op=ALU.mult,
                    )
                    NCHK = 512
                    for j0 in range(0, PPC, NCHK):
                        n = min(NCHK, PPC - j0)
                        psY = psX.tile([OUT, NCHK], F32, tag="py", name="psY")
                        nc.tensor.matmul(
                            out=psY[:, :n], lhsT=W2b[:], rhs=a2s[:, j0 : j0 + n],
                            start=True, stop=True,
                        )
                        ych = sb.tile([OUT, NCHK], F32, tag="ych", name="ych")
                        nc.vector.tensor_tensor(
                            out=ych[:, :n], in0=psY[:, :n],
                            in1=b2c[:].to_broadcast([OUT, n]), op=ALU.add,
                        )
                        nc.sync.dma_start(out=y[:, j0 : j0 + n], in_=ych[:, :n])

            for r in range(reps):
                body(r)

    nc.finalize()
    return nc


def make_in_maps(inputs, sched, idx_list, _colv=None, _dego=None):
    x = np.asarray(inputs["x"], np.float32)
    W1 = np.asarray(inputs["W1"], np.float32)
    W2 = np.asarray(inputs["W2"], np.float32)
    b1 = np.asarray(inputs["b1"], np.float32)
    b2 = np.asarray(inputs["b2"], np.float32)
    dinv = sched["dinv"]
    posmap = sched["posmap"]

    xpre = x * dinv[:, None]
    b1r = np.tile(b1[None, :], (P, 1)).astype(np.float32)
    in_maps = []
    for c in range(NCORES):
        # position-ordered per-core node data
        inv = np.argsort(posmap[c])  # position -> local node
        xs = np.zeros((P, PPC), ml_dtypes.bfloat16)
        xs[:, :TPC] = xpre[c * TPC : (c + 1) * TPC][inv].T.astype(ml_dtypes.bfloat16)
        dpos = np.zeros(PPC, np.float32)
        dpos[:TPC] = dinv[c * TPC : (c + 1) * TPC][inv]
        dsg = np.ascontiguousarray(dpos.reshape(NG, P).T)  # [slot, group]
        drw = np.tile(dpos.astype(ml_dtypes.bfloat16)[None, :], (P, 1))
        in_maps.append(
            {
                "xpT": xs,
                "W1i": W1.astype(ml_dtypes.bfloat16),
                "W2i": W2.astype(ml_dtypes.bfloat16),
                "b1i": b1r,
                "b2i": b2[:, None].astype(np.float32),
                "dsgi": dsg,
                "drwi": drw,
                "idx": idx_list[c],
            }
        )
    return in_maps


def assemble_output(results, sched):
    posmap = sched["posmap"]
    outs = []
    for c in range(NCORES):
        yc = results[c]["y"]  # [OUT, PPC] position-ordered
        outs.append(yc[:, posmap[c]].T)  # [TPC, OUT] node-ordered
    return np.concatenate(outs, axis=0)


def kernel(**inputs):
    sched, idx_list, *_ = host_prep(inputs["edge_index"])
    nc = build_kernel(sched)
    in_maps = make_in_maps(inputs, sched, idx_list)
    res = run_bass_kernel_spmd(nc, in_maps, core_ids=list(range(NCORES)))
    return assemble_output(res.results, sched)
